# revision 9
# baseline (speedup 1.0000x reference)
"""Data2VecVision self-attention Bass kernel for 8 Trainium2 NeuronCores.

Sharding: data-parallel over batch (64 = 8 cores x 8).
Per-core layout strategy:
  - hidden_states shard transposed on host to hsT [768, 8*197] so the
    contraction dim (hidden) lands on SBUF partitions.
  - QT/KT computed as [d_out, s] (fp16), V computed in natural [s, d_out]
    layout padded per-head with a ones column (fp16) so softmax sums fall
    out of the context matmul for free.
  - scores computed transposed [j, i] so the softmax reduction (over j)
    is the matmul contraction dim -> no on-chip transposes anywhere.
  - 1/sqrt(64) folded into Wq/bq on host; V bias bv folded through the
    softmax identity (sum probs == 1) by just keeping bv in V.
Matmul dtypes: fp32r for the three projections (N=394/512 even), fp16 for
the attention matmuls (N=197/65 odd; fp32r forbids odd N).
"""

import numpy as np

import concourse.bacc as bacc
import concourse.mybir as mybir
import concourse.tile as tile
from concourse.bass_utils import run_bass_kernel_spmd

F32 = mybir.dt.float32
F32R = mybir.dt.float32r
F16 = mybir.dt.float16
AF = mybir.ActivationFunctionType

N_CORES = 8
B = 64
NB = B // N_CORES          # batches per core
S = 197
HID = 768
HEADS = 12
D = 64
NHP = HEADS // 2           # head pairs
NCH = HID // 128           # 6 contraction chunks
SB = 2                     # batches per projection block
NBLK = NB // SB            # 4 blocks
SW = SB * S                # 394, projection moving width
CORE_S = NB * S            # 1576
JC = [(0, 128), (128, 69)]   # j/i chunk (offset, len)


def _relative_position_index(h, w):
    coords = np.stack(np.meshgrid(np.arange(h), np.arange(w), indexing="ij")).reshape(2, -1)
    rel = coords[:, :, None] - coords[:, None, :]
    rel = rel.transpose(1, 2, 0).astype(np.int64)
    rel[:, :, 0] += h - 1
    rel[:, :, 1] += w - 1
    rel[:, :, 0] *= 2 * w - 1
    area = h * w
    nrd = (2 * h - 1) * (2 * w - 1) + 3
    idx = np.zeros((area + 1, area + 1), dtype=np.int64)
    idx[1:, 1:] = rel.sum(-1)
    idx[0, :] = nrd - 3
    idx[:, 0] = nrd - 2
    idx[0, 0] = nrd - 1
    return idx


def build_nc(reps=1):
    nc = bacc.Bacc("TRN2", target_bir_lowering=False, debug=False)

    hsT_d = nc.dram_tensor("hsT", [NCH, 128, CORE_S], F32R, kind="ExternalInput").ap()
    wq_d = nc.dram_tensor("wqT", [NCH, 128, HID], F32R, kind="ExternalInput").ap()
    wk_d = nc.dram_tensor("wkT", [NCH, 128, HID], F32R, kind="ExternalInput").ap()
    wv_d = nc.dram_tensor("wvT", [NCH, 128, HID], F32R, kind="ExternalInput").ap()
    bq_d = nc.dram_tensor("bqc", [NCH, 128, 1], F32, kind="ExternalInput").ap()
    bv_d = nc.dram_tensor("bvb", [128, HID], F32, kind="ExternalInput").ap()
    bias_d = nc.dram_tensor("biasp", [HEADS, 2, 128, S], F32, kind="ExternalInput").ap()
    y_d = nc.dram_tensor("y", [NB, S, HID], F32, kind="ExternalOutput").ap()

    with tile.TileContext(nc) as tc:
        with (
            tc.tile_pool(name="res", bufs=1) as res,
            tc.tile_pool(name="vpad", bufs=NB * 2) as vpad_pool,
            tc.tile_pool(name="qt", bufs=2) as qt_pool,
            tc.tile_pool(name="kt", bufs=2) as kt_pool,
            tc.tile_pool(name="st", bufs=3) as st_pool,
            tc.tile_pool(name="et", bufs=3) as et_pool,
            tc.tile_pool(name="rt", bufs=4) as rt_pool,
            tc.tile_pool(name="ot", bufs=4) as ot_pool,
            tc.tile_pool(name="pp", bufs=2, space="PSUM") as proj_ps,
            tc.tile_pool(name="sp", bufs=4, space="PSUM") as sc_ps,
            tc.tile_pool(name="cp", bufs=2, space="PSUM") as ctx_ps,
        ):
            # resident SBUF tensors
            hs_sb = res.tile([128, NCH * CORE_S], F32R)
            wq_sb = res.tile([128, NCH * HID], F32R)
            wk_sb = res.tile([128, NCH * HID], F32R)
            wv_sb = res.tile([128, NCH * HID], F32R)
            bq_sb = res.tile([128, NCH], F32)
            bv_sb = res.tile([128, HID], F32)
            bias_sb = res.tile([128, NHP * 2 * 2 * S], F32)
            vpad = [[vpad_pool.tile([128, HEADS * 65], F16, tag="vp",
                                    name=f"vpad_{b}_{j}") for j in range(2)]
                    for b in range(NB)]

            for _ in range(reps):
                # ---- input DMAs ----
                for c in range(NCH):
                    nc.sync.dma_start(hs_sb[:, c * CORE_S:(c + 1) * CORE_S], hsT_d[c])
                    nc.sync.dma_start(wq_sb[:, c * HID:(c + 1) * HID], wq_d[c])
                    nc.sync.dma_start(wk_sb[:, c * HID:(c + 1) * HID], wk_d[c])
                    nc.sync.dma_start(wv_sb[:, c * HID:(c + 1) * HID], wv_d[c])
                    nc.sync.dma_start(bq_sb[:, c:c + 1], bq_d[c])
                nc.sync.dma_start(bv_sb[:], bv_d[:])
                for g in range(HEADS):
                    for jc in range(2):
                        nc.sync.dma_start(
                            bias_sb[:, (g * 2 + jc) * S:(g * 2 + jc + 1) * S],
                            bias_d[g, jc])

                # ---- V projection (natural [s, d_out] layout, per (b, jc) chunk) ----
                for b in range(NB):
                    for jci, (joff, jlen) in enumerate(JC):
                        vt = vpad[b][jci]
                        ones_ap = vt[:jlen].rearrange("p (h c) -> p h c", h=HEADS)[:, :, 64:65]
                        nc.vector.memset(ones_ap, 1.0)
                        scol = b * S + joff
                        for nt, (noff, nlen) in enumerate([(0, 512), (512, 256)]):
                            vp = proj_ps.tile([128, 512], F32, tag="pp")
                            for c in range(NCH):
                                nc.tensor.matmul(
                                    vp[:jlen, :nlen],
                                    hs_sb[:, c * CORE_S + scol: c * CORE_S + scol + jlen],
                                    wv_sb[:, c * HID + noff: c * HID + noff + nlen],
                                    start=(c == 0), stop=(c == NCH - 1))
                            # bv add + scatter into per-head 65-wide slots
                            dst = vt[:jlen, nt * 8 * 65:].rearrange(
                                "p (h c) -> p h c", c=65)[:, :nlen // 64, :64]
                            nc.vector.tensor_tensor(
                                out=dst, in0=vp[:jlen, :nlen],
                                in1=bv_sb[:jlen, noff:noff + nlen],
                                op=mybir.AluOpType.add)

                # ---- per-block QK projection + attention ----
                for blk in range(NBLK):
                    soff = blk * SW
                    qt = qt_pool.tile([128, NCH * SW], F16, tag="qt")
                    kt = kt_pool.tile([128, NCH * SW], F16, tag="kt")
                    for c in range(NCH):
                        qp = proj_ps.tile([128, SW], F32, tag="pp")
                        for hch in range(NCH):
                            nc.tensor.matmul(
                                qp[:], wq_sb[:, hch * HID + c * 128: hch * HID + (c + 1) * 128],
                                hs_sb[:, hch * CORE_S + soff: hch * CORE_S + soff + SW],
                                start=(hch == 0), stop=(hch == NCH - 1))
                        nc.scalar.activation(qt[:, c * SW:(c + 1) * SW], qp[:],
                                             AF.Identity, bias=bq_sb[:, c:c + 1])
                        kp = proj_ps.tile([128, SW], F32, tag="pp")
                        for hch in range(NCH):
                            nc.tensor.matmul(
                                kp[:], wk_sb[:, hch * HID + c * 128: hch * HID + (c + 1) * 128],
                                hs_sb[:, hch * CORE_S + soff: hch * CORE_S + soff + SW],
                                start=(hch == 0), stop=(hch == NCH - 1))
                        nc.vector.tensor_copy(kt[:, c * SW:(c + 1) * SW], kp[:])

                    for bi in range(SB):
                        b = blk * SB + bi
                        ot = [ot_pool.tile([128, HID], F32, tag="ot",
                                           name=f"ot_{b}_{i}") for i in range(2)]
                        for hp in range(NHP):
                            ets = [[None, None], [None, None]]
                            for jci, (joff, jlen) in enumerate(JC):
                                for h in range(2):
                                    g = hp * 2 + h
                                    sp = sc_ps.tile([128, S], F32, tag="sp")
                                    col = hp * SW + bi * S
                                    nc.tensor.matmul(
                                        sp[:jlen],
                                        kt[h * 64:(h + 1) * 64, col + joff: col + joff + jlen],
                                        qt[h * 64:(h + 1) * 64, col: col + S],
                                        start=True, stop=True)
                                    st = st_pool.tile([128, S], F16, tag="st")
                                    nc.vector.tensor_tensor(
                                        out=st[:jlen], in0=sp[:jlen],
                                        in1=bias_sb[:jlen, (g * 2 + jci) * S:
                                                    (g * 2 + jci + 1) * S],
                                        op=mybir.AluOpType.add)
                                    et = et_pool.tile([128, S], F16, tag="et")
                                    nc.scalar.activation(et[:jlen], st[:jlen], AF.Exp)
                                    ets[jci][h] = et
                            for ici, (ioff, ilen) in enumerate(JC):
                                cp = ctx_ps.tile([128, 130], F32, tag="cp")
                                for h in range(2):
                                    for jci, (joff, jlen) in enumerate(JC):
                                        nc.tensor.matmul(
                                            cp[:ilen, h * 65:(h + 1) * 65],
                                            ets[jci][h][:jlen, ioff: ioff + ilen],
                                            vpad[b][jci][:jlen, (hp * 2 + h) * 65:
                                                         (hp * 2 + h + 1) * 65],
                                            start=(jci == 0), stop=(jci == 1))
                                r = rt_pool.tile([128, 2], F32, tag="rt")
                                sums = cp[:ilen].rearrange("p (h c) -> p h c", c=65)[:, :, 64:65]
                                nc.vector.reciprocal(r[:ilen], sums)
                                for h in range(2):
                                    nc.scalar.activation(
                                        ot[ici][:ilen, (hp * 2 + h) * 64:(hp * 2 + h + 1) * 64],
                                        cp[:ilen, h * 65: h * 65 + 64],
                                        AF.Copy, scale=r[:ilen, h:h + 1])
                        for ici, (ioff, ilen) in enumerate(JC):
                            nc.sync.dma_start(y_d[b, ioff:ioff + ilen, :], ot[ici][:ilen])

    nc.compile()
    return nc


_NC_CACHE = {}


def _get_nc(reps=1):
    if reps not in _NC_CACHE:
        _NC_CACHE[reps] = build_nc(reps)
    return _NC_CACHE[reps]


def prep_inputs(hidden_states, Wq, bq, Wk, Wv, bv, bias_table):
    hidden_states = np.asarray(hidden_states, np.float32)
    Wq = np.asarray(Wq, np.float32)
    bq = np.asarray(bq, np.float32)
    Wk = np.asarray(Wk, np.float32)
    Wv = np.asarray(Wv, np.float32)
    bv = np.asarray(bv, np.float32)
    bias_table = np.asarray(bias_table, np.float32)

    wqT = np.ascontiguousarray((Wq / 8.0).T).reshape(NCH, 128, HID)
    wkT = np.ascontiguousarray(Wk.T).reshape(NCH, 128, HID)
    wvT = np.ascontiguousarray(Wv.T).reshape(NCH, 128, HID)
    bqc = (bq / 8.0).astype(np.float32).reshape(NCH, 128, 1)
    bvb = np.ascontiguousarray(np.broadcast_to(bv, (128, HID))).astype(np.float32)

    idx = _relative_position_index(14, 14)
    bias_full = bias_table[idx]              # [S, S, HEADS] (i, j, h)
    biasT = bias_full.transpose(2, 1, 0)     # [h, j, i]
    biasp = np.zeros((HEADS, 2, 128, S), np.float32)
    for g in range(HEADS):
        for jci, (joff, jlen) in enumerate(JC):
            biasp[g, jci, :jlen, :] = biasT[g, joff:joff + jlen, :]

    shared = {"wqT": wqT, "wkT": wkT, "wvT": wvT, "bqc": bqc, "bvb": bvb,
              "biasp": biasp}
    in_maps = []
    for c in range(N_CORES):
        hs_c = hidden_states[c * NB:(c + 1) * NB]            # [NB, S, HID]
        hsT = np.ascontiguousarray(hs_c.transpose(2, 0, 1).reshape(HID, CORE_S))
        in_maps.append({"hsT": hsT.reshape(NCH, 128, CORE_S), **shared})
    return in_maps


def run(in_maps, reps=1):
    nc = _get_nc(reps)
    res = run_bass_kernel_spmd(nc, in_maps, core_ids=list(range(N_CORES)))
    return np.concatenate([res.results[c]["y"] for c in range(N_CORES)], axis=0)


def kernel(hidden_states, Wq, bq, Wk, Wv, bv, bias_table,
           resolution_h=224, resolution_w=224):
    in_maps = prep_inputs(hidden_states, Wq, bq, Wk, Wv, bv, bias_table)
    return run(in_maps, reps=1)
